# revision 2
# baseline (speedup 1.0000x reference)
"""NgramHasher Trainium2 kernel, v3.

Computes h[b,s,ch] = (sum_j coeffs[k,j] * window_j) mod 2^20 for ngram sizes
(2, 3) x 8 tables, on 8 NeuronCores (data parallel over batch).

Math: with c = c0 + 2^10*c1 and t = t0 + 2^10*t1 (chunks 10/8-bit),
  h = (A + 2^10 * (B mod 2^10)) mod 2^20
  A = sum_j c0[j]*t0[s-j]                     (fp16 matmul, exact in fp32)
  B = sum_j (c0[j]*t1[s-j] + c1[j]*t0[s-j])   (fp16 matmul + 2^23 bias row)

The 2^23 bias pins PSUM_B's fp32 exponent so its low mantissa bits ARE
B mod 1024.  Extraction m = (bits & 0x3FF) << 10 is a DVE tensor_scalar
(all-bitwise, dtype-preserving); the merge o = A + m is a tensor_tensor
add producing uint32 in SBUF; host masks with 0xFFFFF.

Engine balance (PE capped at 1.2 GHz on this platform => 2 passes over
[128, 8192] = 13.7us is the wall; everything else must hide under it):
  - "dve"  chunks: DVE TS on PSUM_B (1x) + DVE TT(ps_a, m)
  - "dveb" chunks: ACT copy PSUM_B->SBUF u32, DVE TS on SBUF (2x mode),
                   DVE TT(ps_a, m)
  - "gps"  chunks: DVE TS on PSUM_B + ACT copy PSUM_A->u32 + GPSIMD
                   integer TT (Pool cannot read PSUM, needs all-u32)
Chunks taper to 512 cols at the end ("dve" mode, output DMA split across
both HWDGE queues) to shorten the serial drain chain.  Inputs stream in
4 pieces across the sync+scalar queues so the first matmul starts early.

Matmul packing: M = 128 = 8 position-offsets (g) x 16 channels; moving
columns are position groups q (position s = 8q + g); weights are banded
Toeplitz.  Device output is [128, 8192] uint32 per core; the host
unshards/permutes to [64, 8192, 16] int64.
"""
import sys
sys.path.insert(0, "/opt/trn_rl_repo")
import numpy as np
from contextlib import ExitStack
from numpy.lib.stride_tricks import sliding_window_view

import concourse.bass as bass
import concourse.tile as tile
from concourse import bacc, mybir
from concourse.bass_utils import run_bass_kernel_spmd

dt = mybir.dt
AluOp = mybir.AluOpType

N_CORES = 8
B, S = 64, 8192
B_LOC = B // N_CORES            # batch rows per core
P_CORE = B_LOC * S              # positions per core (65536)
G = 8                           # position offsets packed into M
NCH = 16                        # output channels (2 ngram sizes x 8 tables)
Q = P_CORE // G                 # moving columns per core (8192)

CHUNK_COLS = [1024] * 7 + [512, 512]
MERGE_PLAN = ["gps", "gps", "dve", "gps", "dve", "gps", "dve", "pe", "pe"]
# Output DMA queue per chunk; "split" = halves on sync+scalar concurrently.
DMA_PLAN = ["sync", "scalar", "sync", "scalar", "sync", "scalar", "sync",
            "sync", "scalar"]
# Chunks whose merge TT + output DMA are emitted after all other chunks,
# so the tail chunks' extraction/merge ops are not queued behind them.
DEFER_TT = []
# Input arrives in pieces: (start_col, n_cols, queue)
IN_PLAN = [(0, 1024, "scalar"), (1024, 1024, "scalar"),
           (2048, 2048, "sync"), (4096, 4096, "sync")]

_NC_CACHE = {}


def _emit_out_dma(nc, out_d, dq, q0, cols, o):
    if dq == "split":
        half = cols // 2
        nc.sync.dma_start(out_d[:, q0:q0 + half], o[:, :half])
        nc.scalar.dma_start(out_d[:, q0 + half:q0 + cols], o[:, half:])
    else:
        eng = {"sync": nc.sync, "scalar": nc.scalar,
               "gpsimd": nc.gpsimd}[dq]
        eng.dma_start(out_d[:, q0:q0 + cols], o[:])


def _build_bass():
    """Build the SPMD Bass program (identical on all 8 cores)."""
    assert sum(CHUNK_COLS) == Q
    nchunk = len(CHUNK_COLS)
    use_pe = "pe" in MERGE_PLAN
    nc = bacc.Bacc("TRN2", target_bir_lowering=False, debug=False,
                   num_devices=N_CORES)
    x_d = nc.dram_tensor("X", [21, Q], dt.float16, kind="ExternalInput").ap()
    wb_d = nc.dram_tensor("WB", [21, 128], dt.float16, kind="ExternalInput").ap()
    wa_d = nc.dram_tensor("WA", [21, 128], dt.float16, kind="ExternalInput").ap()
    if use_pe:
        wi_d = nc.dram_tensor("WI", [128, 128], dt.float16,
                              kind="ExternalInput").ap()
    out_d = nc.dram_tensor("OUT", [128, Q], dt.uint32, kind="ExternalOutput").ap()

    with tile.TileContext(nc) as tc:
        with ExitStack() as ctx:
            # Over-sized bufs: no SBUF slot reuse -> DMAs carry no WAR waits.
            wpool = ctx.enter_context(tc.tile_pool(name="w", bufs=1))
            xpool = ctx.enter_context(tc.tile_pool(name="x", bufs=1))
            mpool = ctx.enter_context(tc.tile_pool(name="m", bufs=nchunk + 1))
            opool = ctx.enter_context(tc.tile_pool(name="o", bufs=nchunk + 1))
            psa = ctx.enter_context(tc.tile_pool(name="psa", bufs=2, space="PSUM"))
            psb = ctx.enter_context(tc.tile_pool(name="psb", bufs=2, space="PSUM"))

            # Weights first per queue (tiny, they gate every matmul), then
            # input pieces interleaved across both HWDGE queues.
            w_a = wpool.tile([21, 128], dt.float16, tag="wa")
            nc.sync.dma_start(w_a[:], wa_d[:])
            w_b = wpool.tile([21, 128], dt.float16, tag="wb")
            nc.sync.dma_start(w_b[:], wb_d[:])
            if use_pe:
                w_i = wpool.tile([128, 128], dt.float16, tag="wi")
                nc.sync.dma_start(w_i[:], wi_d[:])

            xts = {}
            for pi, (col0, ncols, queue) in enumerate(IN_PLAN):
                xt = xpool.tile([21, ncols], dt.float16, tag=f"xt{pi}",
                                name=f"xt{pi}")
                eng = nc.sync if queue == "sync" else nc.scalar
                eng.dma_start(xt[:], x_d[:, col0:col0 + ncols])
                xts[col0] = (xt, ncols)

            def x_slice(q0, cols):
                for col0, (xt, ncols) in xts.items():
                    if col0 <= q0 and q0 + cols <= col0 + ncols:
                        return xt[:, q0 - col0:q0 - col0 + cols]
                raise AssertionError(f"chunk [{q0},{q0+cols}) splits input pieces")

            q0 = 0
            deferred = []
            for ci, cols in enumerate(CHUNK_COLS):
                merge = MERGE_PLAN[ci]
                xc = x_slice(q0, cols)

                ps_b = psb.tile([128, 1024], dt.float32, tag="psb")
                ps_a = psa.tile([128, 1024], dt.float32, tag="psa")

                for h in range(cols // 512):
                    c0, c1 = h * 512, (h + 1) * 512
                    nc.tensor.matmul(ps_a[:, c0:c1], w_a[:], xc[:, c0:c1],
                                     start=True, stop=(merge != "pe"))
                for h in range(cols // 512):
                    c0, c1 = h * 512, (h + 1) * 512
                    nc.tensor.matmul(ps_b[:, c0:c1], w_b[:], xc[:, c0:c1],
                                     start=True, stop=True)

                o = opool.tile([128, cols], dt.uint32, tag="o", name=f"o{ci}")
                if merge == "pe":
                    # u16 fp16 pattern with value 1024 + (B mod 1024)
                    u = mpool.tile([128, cols], dt.uint16, tag="m", name=f"u{ci}")
                    nc.vector.tensor_scalar(
                        u[:], ps_b[:, :cols].bitcast(dt.uint16)[:, ::2],
                        0x3FF, 25 << 10, AluOp.bitwise_and, AluOp.bitwise_or)
                    for h in range(cols // 512):
                        c0, c1 = h * 512, (h + 1) * 512
                        nc.tensor.matmul(ps_a[:, c0:c1], w_i[:],
                                         u[:, c0:c1].bitcast(dt.float16),
                                         start=False, stop=True)
                    nc.scalar.copy(o[:], ps_a[:, :cols])
                else:
                    m = mpool.tile([128, cols], dt.uint32, tag="m", name=f"m{ci}")
                    nc.vector.tensor_scalar(
                        m[:], ps_b[:, :cols].bitcast(dt.uint32), 0x3FF, 10,
                        AluOp.bitwise_and, AluOp.logical_shift_left)
                    if merge in ("gps", "gpsd"):
                        # ACT copy releases PSUM_A promptly; Pool cannot read
                        # PSUM and needs all-matching dtypes anyway.
                        a = mpool.tile([128, cols], dt.uint32, tag="a",
                                       name=f"a{ci}")
                        nc.scalar.copy(a[:], ps_a[:, :cols])
                        if merge == "gps":
                            nc.gpsimd.tensor_tensor(o[:], a[:], m[:],
                                                    AluOp.add)
                        elif ci in DEFER_TT:
                            deferred.append((ci, q0, cols, a, m, o))
                        else:
                            nc.vector.tensor_tensor(o[:], a[:], m[:],
                                                    AluOp.add)
                    else:
                        nc.vector.tensor_tensor(o[:], ps_a[:, :cols], m[:],
                                                AluOp.add)

                if ci not in DEFER_TT:
                    _emit_out_dma(nc, out_d, DMA_PLAN[ci], q0, cols, o)
                q0 += cols

            for ci, cq0, cols, a, m, o in deferred:
                nc.vector.tensor_tensor(o[:], a[:], m[:], AluOp.add)
                _emit_out_dma(nc, out_d, DMA_PLAN[ci], cq0, cols, o)
    nc.compile()
    return nc


def _get_nc():
    if "nc" not in _NC_CACHE:
        _NC_CACHE["nc"] = _build_bass()
    return _NC_CACHE["nc"]


def _band(cpart):
    """[8,3] coeff chunk -> banded Toeplitz [10, 128] weight (fp32 values)."""
    W = np.zeros((10, 128), np.float32)
    for g in range(G):
        for k in range(8):
            for j in range(2):              # ngram n=2 -> channels 0..7
                W[g + 1 + j, g * 16 + k] = cpart[k, j]
            for j in range(3):              # ngram n=3 -> channels 8..15
                W[g + j, g * 16 + 8 + k] = cpart[k, j]
    return W


def _host_prep(token_ids, coeffs):
    t = np.asarray(token_ids).astype(np.int64)
    c = np.asarray(coeffs).astype(np.int64)

    t0 = (t & 0x3FF).astype(np.float16)     # [64, 8192]
    t1 = (t >> 10).astype(np.float16)
    pad = np.zeros((B, 2), np.float16)
    t0p = np.concatenate([pad, t0], axis=1)  # [64, 8194]
    t1p = np.concatenate([pad, t1], axis=1)
    # w?[b, q_loc, r] = t?p[b, 8*q_loc + r],  q_loc in [0,1024), r in [0,10)
    w0 = sliding_window_view(t0p, 10, axis=1)[:, ::G, :]
    w1 = sliding_window_view(t1p, 10, axis=1)[:, ::G, :]
    w0 = np.ascontiguousarray(w0.transpose(0, 2, 1))  # [64, 10, 1024]
    w1 = np.ascontiguousarray(w1.transpose(0, 2, 1))

    c0 = (c & 0x3FF).astype(np.float32)
    c1 = (c >> 10).astype(np.float32)
    # 2^23 bias arrives as (2^15 weight) * (2^8 const input row): fp16-exact
    bias_row = np.full((1, 128), float(1 << 15), np.float32)
    WB = np.concatenate([_band(c0), _band(c1), bias_row],
                        axis=0).astype(np.float16)
    WA = np.concatenate([np.zeros((10, 128), np.float32), _band(c0),
                         np.zeros((1, 128), np.float32)],
                        axis=0).astype(np.float16)

    in_maps = []
    for core in range(N_CORES):
        b0 = core * B_LOC
        X = np.empty((21, Q), np.float16)
        # rows 0..9: X1 windows; rows 10..19: X0 windows; row 20: const
        X[0:10] = w1[b0:b0 + B_LOC].transpose(1, 0, 2).reshape(10, Q)
        X[10:20] = w0[b0:b0 + B_LOC].transpose(1, 0, 2).reshape(10, Q)
        X[20] = 256.0
        im = {"X": X, "WB": WB, "WA": WA}
        if "pe" in MERGE_PLAN:
            im["WI"] = (1024.0 * np.eye(128)).astype(np.float16)
        in_maps.append(im)
    return in_maps


def _unshard(results):
    out = np.empty((B, S, NCH), np.int64)
    for core, res in enumerate(results):
        o = (res["OUT"] & 0xFFFFF).reshape(G, NCH, Q)  # [g, ch, q]
        o = o.transpose(2, 0, 1).reshape(P_CORE, NCH)  # [8q+g, ch]
        out[core * B_LOC:(core + 1) * B_LOC] = \
            o.reshape(B_LOC, S, NCH).astype(np.int64)
    return out


def _run(token_ids, coeffs, **spmd_kwargs):
    in_maps = _host_prep(token_ids, coeffs)
    nc = _get_nc()
    res = run_bass_kernel_spmd(nc, in_maps, core_ids=list(range(N_CORES)),
                               **spmd_kwargs)
    return _unshard(res.results), res


def kernel(token_ids, coeffs):
    out, _ = _run(token_ids, coeffs)
    return out
